# revision 14
# baseline (speedup 1.0000x reference)
"""Trainium2 Bass kernel for nn_MaskConv (2x masked conv+bn+hardtanh stages).

Data-parallel over batch: 16 images -> 8 cores, 2 images/core.

Per-core device program (SPMD, no collectives):
  conv1: 1->32ch, 41x11 kernel, stride (2,2), input padded host-side to
         [201,1024]. im2col row-window DMAs ([41,1024] windows, 8 windows
         per batch at partition bases 0 and 64), matmul K=41(kh), M=32(co),
         kw folded into 11 PSUM column-shifted accumulating matmuls
         (stride-2 rhs APs). 2 row-groups x 4 col-groups tile packing.
  bn1+hardtanh+mask: folded to Relu(psum*S1+T1) on ACT then
         min(., mask20) on DVE (mask20 = 20.0 where t<len else 0.0).
  h1 layout: two phase copies, partition p = 32*((row-off)%4)+ci,
         free q = (row-off)//4, off in {0,-2} -> any conv2 K=128 window
         (4 rows x 32 ci) is a 32-aligned partition slice.
  conv2: 32->32ch, 21x11, stride (2,1). Per output row: ~6 kh-blocks
         (K=128) x 11 kw PSUM-shifted matmuls, 4 col-tiles = 4 rows/batch.
  bn2+hardtanh+mask epilogue, DMA to DRAM out.

Matmul dtype mode via env KMODE: 'f32' (exact, 4cy/row), 'f32r'
(full-rate fp32-reduced), 'bf16'. Default f32r.
"""

import os
import sys

import numpy as np

sys.path.insert(0, "/opt/trn_rl_repo")

import concourse.bass as bass  # noqa: E402
import concourse.mybir as mybir  # noqa: E402
import concourse.tile as tile  # noqa: E402
from concourse.tile_rust import add_dep_helper  # noqa: E402
from concourse.bass_utils import run_bass_kernel_spmd  # noqa: E402

N_CORES = 8
IMGS_PER_CORE = 2
H, W = 161, 1024
HP = H + 40  # host-padded rows
OH1, OW1 = 81, 512  # conv1 output
OH2, OW2 = 41, 512  # conv2 output
EPS = 1e-5

MODE = os.environ.get("KMODE", "f32r")

if MODE == "bf16":
    import ml_dtypes

    NP_STORE = ml_dtypes.bfloat16
    DT_STORE = mybir.dt.bfloat16
    DT_MM = mybir.dt.bfloat16
else:
    NP_STORE = np.float32
    DT_STORE = mybir.dt.float32
    DT_MM = mybir.dt.float32r if MODE == "f32r" else mybir.dt.float32

F32 = mybir.dt.float32

# conv1 batches: 8 output rows each (4 at partition base 0, 4 at base 64)
C1_STARTS = list(range(0, 73, 8)) + [73]  # 11 batches, last overlaps
# conv2 batches: 4 output rows each
C2_STARTS = list(range(0, 37, 4)) + [37]  # 11 batches

# h1 phase copies: (row offset, q slots)
H1_COPIES = {0: (0, 21), 2: (-2, 22)}


def _mm_ap(ap):
    if DT_MM is mybir.dt.float32r:
        return ap.bitcast(mybir.dt.float32r)
    return ap


def _c1_kw_geom(kw):
    """Return (parity, d, lo, hi) for conv1 tap kw: x col = 2(ow+d)+parity."""
    if kw % 2 == 1:
        parity, d = 0, (kw - 5) // 2
    else:
        parity, d = 1, (kw - 6) // 2
    lo = max(0, -d)
    hi = min(OW1, OW1 - d)
    return parity, d, lo, hi


def _c2_b_range(row):
    """Valid kh-block range for conv2 output row (uses phase copy by parity)."""
    phi = 2 if row % 2 == 0 else 0
    off, nq = H1_COPIES[phi]
    # window rows [s, s+4) must lie in [off, off+4*nq); s = 2*row-10+4*b
    b_lo = max(0, -((-(off + 10 - 2 * row)) // 4))  # ceil((off+10-2row)/4)
    b_hi = min(6, (off + 4 * nq - 4 - (2 * row - 10)) // 4 + 1)
    return phi, off, b_lo, b_hi


def _h1_runs(r0):
    """For 4 consecutive conv1 rows starting at r0, compute per-copy DMA runs.

    Returns list of (phi, stg_block_start, n_blocks, q, h1_part_base)."""
    runs = []
    for phi, (off, _nq) in H1_COPIES.items():
        i = 0
        while i < 4:
            r = r0 + i
            u = r - off
            q, pb = u // 4, 32 * (u % 4)
            n = 1
            while i + n < 4 and (u + n) % 4 != 0:
                n += 1
            runs.append((phi, i, n, q, pb))
            i += n
    return runs


def build_nc():
    nc = bass.Bass()

    xp_t = nc.dram_tensor("xp", [IMGS_PER_CORE, HP, W], DT_STORE, kind="ExternalInput")
    w1_t = nc.dram_tensor("w1l", [128, 11, 32], DT_STORE, kind="ExternalInput")
    w2_t = nc.dram_tensor("w2l", [128, 6, 11, 32], DT_STORE, kind="ExternalInput")
    mask_t = nc.dram_tensor("mask20", [IMGS_PER_CORE, 128, OW1], F32, kind="ExternalInput")
    s1_t = nc.dram_tensor("s1", [128, 1], F32, kind="ExternalInput")
    t1_t = nc.dram_tensor("t1", [128, 1], F32, kind="ExternalInput")
    s2_t = nc.dram_tensor("s2", [128, 1], F32, kind="ExternalInput")
    t2_t = nc.dram_tensor("t2", [128, 1], F32, kind="ExternalInput")
    out_t = nc.dram_tensor("out", [IMGS_PER_CORE, 32, OH2, OW2], F32, kind="ExternalOutput")

    esz = mybir.dt.size(DT_STORE)

    with tile.TileContext(nc) as tc:
        with (
            tc.tile_pool(name="wpool", bufs=1) as wpool,
            tc.tile_pool(name="vecs", bufs=1) as vecs,
            tc.tile_pool(name="maskp", bufs=2) as maskp,
            tc.tile_pool(name="h1p", bufs=1) as h1p,
            tc.tile_pool(name="imp", bufs=3) as imp,
            tc.tile_pool(name="stgp", bufs=8) as stgp,
            tc.tile_pool(name="ps1", bufs=4, space="PSUM") as ps1,
            tc.tile_pool(name="ps2", bufs=3, space="PSUM") as ps2,
            tc.tile_pool(name="pssc", bufs=1, space="PSUM") as pssc,
        ):
            # Wait-laundering: walrus caps the number of embedded sem-waits
            # on a Matmult (LDWEIGHTS slot). Tiny "absorber" matmuls into a
            # scratch psum bank carry the DMA-queue waits on PE right after
            # each producer, so real matmuls keep <=2 embedded waits.
            scr = pssc.tile([128, OW2], F32, tag="scr")
            ascr = stgp.tile([128, 4], F32, tag="ascr", bufs=1)
            dscr = stgp.tile([128, 4], F32, tag="dscr", bufs=1)

            pe_abs_now = []      # chain to next real matmul
            pe_abs_img = []      # chain to first conv2 matmul of the image
            dve_abs_pending = [] # chain to next real DVE op
            act_abs_pending = [] # chain to next real ACT op

            def _absorb_pe(src_col, tp0, dest):
                i = nc.tensor.matmul(
                    scr[0:1, 0:1], _mm_ap(src_col), _mm_ap(src_col),
                    start=True, stop=True, tile_position=(tp0, 0),
                    skip_group_check=True,
                )
                dest.append(i.ins)

            def _absorb_act(src_11):
                i = nc.scalar.activation(
                    ascr[0:1, 0:1], src_11, mybir.ActivationFunctionType.Copy
                )
                act_abs_pending.append(i.ins)

            def _absorb_dve(src_11, pb=0):
                i = nc.vector.tensor_copy(dscr[pb : pb + 1, 0:1], src_11)
                dve_abs_pending.append(i.ins)

            def _chain(real_bass_inst, pending):
                for a in pending:
                    add_dep_helper(real_bass_inst.ins, a, sync=False,
                                   reason="order wait-absorber first")
                pending.clear()
                return real_bass_inst

            w1l = wpool.tile([128, 11, 32], DT_STORE, tag="w1l")
            w2l = wpool.tile([128, 6, 11, 32], DT_STORE, tag="w2l")
            nc.sync.dma_start(w1l[:], w1_t[:])
            nc.sync.dma_start(w2l[:], w2_t[:])
            _absorb_pe(w1l[0:41, 0, 0:1], 0, pe_abs_now)
            _absorb_pe(w1l[64:105, 0, 0:1], 64, pe_abs_now)
            _absorb_pe(w2l[0:128, 0, 0, 0:1], 0, pe_abs_now)
            s1v = vecs.tile([128, 1], F32, tag="s1")
            t1v = vecs.tile([128, 1], F32, tag="t1")
            s2v = vecs.tile([128, 1], F32, tag="s2")
            t2v = vecs.tile([128, 1], F32, tag="t2")
            nc.sync.dma_start(s1v[:], s1_t[:])
            nc.sync.dma_start(t1v[:], t1_t[:])
            nc.sync.dma_start(s2v[:], s2_t[:])
            nc.sync.dma_start(t2v[:], t2_t[:])
            for v in (s1v, t1v, s2v, t2v):
                _absorb_act(v[0:1, 0:1])

            for img in range(IMGS_PER_CORE):
                mk = maskp.tile([128, OW1], F32, tag="mask")
                nc.sync.dma_start(mk[:], mask_t[img])
                _absorb_dve(mk[0:1, 0:1])

                h1 = {
                    0: h1p.tile([128, 21, OW1], DT_STORE, tag="h1p0", name=f"h1p0_{img}"),
                    2: h1p.tile([128, 22, OW1], DT_STORE, tag="h1p2", name=f"h1p2_{img}"),
                }
                # zero pad slots (rows outside [0,81)); DVE ops with base
                # partition > 0 may span at most 32 partitions -> chunk.
                def _memset32(t, p0, p1, q):
                    for p in range(p0, p1, 32):
                        nc.vector.memset(t[p : p + 32, q, :], 0.0)

                def _memset_absorb(t, p0, p1, q):
                    _memset32(t, p0, p1, q)
                    for p in range(p0, p1, 32):
                        _absorb_pe(t[p : p + 32, q, 0:1], p, pe_abs_img)

                _memset_absorb(h1[0], 32, 128, 20)
                _memset_absorb(h1[2], 0, 64, 0)
                _memset_absorb(h1[2], 96, 128, 20)
                _memset_absorb(h1[2], 0, 128, 21)

                # ---------------- conv1 ----------------
                kw_order = [5, 1, 3, 7, 9, 0, 2, 4, 6, 8, 10]
                for oh0 in C1_STARTS:
                    imt = imp.tile([128, 4, W], DT_STORE, tag="imcol")
                    for rg in range(2):
                        src = bass.AP(
                            tensor=xp_t,
                            offset=(img * HP + 2 * (oh0 + 4 * rg)) * W,
                            ap=[[W, 41], [2 * W, 4], [1, W]],
                        )
                        nc.sync.dma_start(imt[64 * rg : 64 * rg + 41, :, :], src)
                        _absorb_pe(imt[64 * rg : 64 * rg + 41, 0, 0:1], 64 * rg, pe_abs_now)
                    imr = imt[:].rearrange("p j (w two) -> p j two w", two=2)
                    banks = [
                        ps1.tile([128, OW1], F32, tag="c1bank", name=f"c1bank0_{img}_{oh0}"),
                        ps1.tile([128, OW1], F32, tag="c1bank", name=f"c1bank1_{img}_{oh0}"),
                    ]
                    for i_kw, kw in enumerate(kw_order):
                        parity, d, lo, hi = _c1_kw_geom(kw)
                        for rg in range(2):
                            for c in range(4):
                                _chain(nc.tensor.matmul(
                                    banks[rg][32 * c : 32 * c + 32, lo:hi],
                                    _mm_ap(w1l[64 * rg : 64 * rg + 41, kw, :]),
                                    _mm_ap(imr[64 * rg : 64 * rg + 41, c, parity, lo + d : hi + d]),
                                    start=(i_kw == 0),
                                    stop=(i_kw == len(kw_order) - 1),
                                    tile_position=(64 * rg, 32 * c),
                                    skip_group_check=True,
                                ), pe_abs_now)
                    for rg in range(2):
                        r0 = oh0 + 4 * rg
                        stg = stgp.tile([128, OW1], F32, tag="stg_f32")
                        _chain(nc.scalar.activation(
                            stg[:], banks[rg][:], mybir.ActivationFunctionType.Relu,
                            bias=t1v[:], scale=s1v[:],
                        ), act_abs_pending)
                        stg2 = stgp.tile([128, OW1], DT_STORE, tag="stg_d")
                        _chain(nc.vector.tensor_tensor(
                            stg2[:], stg[:], mk[:], mybir.AluOpType.min
                        ), dve_abs_pending)
                        for phi, i0, nblk, q, pb in _h1_runs(r0):
                            nc.sync.dma_start(
                                h1[phi][pb : pb + 32 * nblk, q, :],
                                stg2[32 * i0 : 32 * (i0 + nblk), :],
                            )
                            for pp in range(pb, pb + 32 * nblk, 32):
                                _absorb_pe(h1[phi][pp : pp + 32, q, 0:1], pp, pe_abs_img)
                                _absorb_dve(h1[phi][pp : pp + 1, q, 0:1], pp)

                # ---------------- conv2 ----------------
                for oh0 in C2_STARTS:
                    bank = ps2.tile([128, OW2], F32, tag="c2bank")
                    mm_lists = []
                    for c in range(4):
                        row = oh0 + c
                        phi, off, b_lo, b_hi = _c2_b_range(row)
                        lst = []
                        for bi, b in enumerate(range(b_lo, b_hi)):
                            kws = [5] + [k for k in range(11) if k != 5] if bi == 0 else list(range(11))
                            for j, kw in enumerate(kws):
                                first = bi == 0 and j == 0
                                last = (b == b_hi - 1) and (j == len(kws) - 1)
                                lst.append((b, kw, phi, off, row, first, last))
                        mm_lists.append(lst)
                    for k in range(max(len(l) for l in mm_lists)):
                        for c in range(4):
                            if k >= len(mm_lists[c]):
                                continue
                            b, kw, phi, off, row, first, last = mm_lists[c][k]
                            s = 2 * row - 10 + 4 * b
                            q = (s - off) // 4
                            d = kw - 5
                            lo = max(0, -d)
                            hi = min(OW2, OW2 - d)
                            mm = nc.tensor.matmul(
                                bank[32 * c : 32 * c + 32, lo:hi],
                                _mm_ap(w2l[:, b, kw, :]),
                                _mm_ap(h1[phi][:, q, lo + d : hi + d]),
                                start=first,
                                stop=last,
                                tile_position=(0, 32 * c),
                                skip_group_check=True,
                            )
                            _chain(mm, pe_abs_now)
                            _chain(mm, pe_abs_img)
                    stg = stgp.tile([128, OW2], F32, tag="stg_f32")
                    _chain(nc.scalar.activation(
                        stg[:], bank[:], mybir.ActivationFunctionType.Relu,
                        bias=t2v[:], scale=s2v[:],
                    ), act_abs_pending)
                    stg2 = stgp.tile([128, OW2], F32, tag="stg_out")
                    _chain(nc.vector.tensor_tensor(
                        stg2[:], stg[:], mk[:], mybir.AluOpType.min
                    ), dve_abs_pending)
                    for c in range(4):
                        dst = bass.AP(
                            tensor=out_t,
                            offset=((img * 32) * OH2 + (oh0 + c)) * OW2,
                            ap=[[OH2 * OW2, 32], [1, OW2]],
                        )
                        nc.sync.dma_start(dst, stg2[32 * c : 32 * c + 32, :])

    _split_excess_waits(nc, max_waits=1)
    return nc


def _split_excess_waits(nc, max_waits=1):
    """Walrus caps embedded sem-waits per instruction (engine-struct
    dependent). Move excess waits onto standalone EventSemaphore
    instructions inserted right before the offender, same engine."""
    fn = nc.m.functions[0]
    cnt = 0
    for blk in fn.blocks:
        il = blk.instructions
        i = 0
        while i < len(il):
            inst = il[i]
            si = inst.sync_info
            n = len(si.on_wait) if si and si.on_wait else 0
            if n > max_waits and type(inst).__name__ != "InstEventSemaphore":
                waits = list(si.on_wait)
                keep, excess = waits[:max_waits], waits[max_waits:]
                for k, w in enumerate(excess):
                    ev = mybir.InstEventSemaphore(name=f"WX-{cnt}", ins=[], outs=[])
                    cnt += 1
                    ev.engine = inst.engine
                    ev.sync_info = mybir.SyncInfo(on_wait=[w], on_update=[])
                    il.insert(i + k, ev)
                inst.sync_info = mybir.SyncInfo(
                    on_wait=keep, on_update=list(si.on_update or [])
                )
                i += len(excess)
            i += 1
    return nc


_NC = None


def _get_nc():
    global _NC
    if _NC is None:
        _NC = build_nc()
    return _NC


def _prep_core_inputs(x, x_lengths, w1, b1, g1, be1, m1, v1, w2, b2, g2, be2, m2, v2):
    """Host-side prep shared across cores (weights) + per-core shards."""
    s1 = (g1 / np.sqrt(v1 + EPS)).astype(np.float32)
    t1 = ((b1 - m1) * s1 + be1).astype(np.float32)
    s2 = (g2 / np.sqrt(v2 + EPS)).astype(np.float32)
    t2 = ((b2 - m2) * s2 + be2).astype(np.float32)

    w1l = np.zeros((128, 11, 32), np.float32)
    w1t = np.transpose(w1[:, 0], (1, 2, 0))  # [41, 11, 32]
    w1l[0:41] = w1t
    w1l[64:105] = w1t

    # w2l[32*dh+ci, b, kw, co] = w2[co, ci, 4b+dh, kw]
    w2l = np.zeros((128, 6, 11, 32), np.float32)
    for dh in range(4):
        for b in range(6):
            kh = 4 * b + dh
            if kh < 21:
                w2l[32 * dh : 32 * dh + 32, b] = np.transpose(w2[:, :, kh, :], (1, 2, 0))

    s1t = np.tile(s1, 4)[:, None]
    t1t = np.tile(t1, 4)[:, None]
    s2t = np.tile(s2, 4)[:, None]
    t2t = np.tile(t2, 4)[:, None]

    lengths = np.asarray(x_lengths).astype(np.int64)
    iota = np.arange(OW1)

    in_maps = []
    for core in range(N_CORES):
        sl = slice(core * IMGS_PER_CORE, (core + 1) * IMGS_PER_CORE)
        xs = np.asarray(x[sl, 0], np.float32)
        xp = np.pad(xs, ((0, 0), (20, 20), (0, 0))).astype(NP_STORE)
        m20 = np.where(iota[None, :] < lengths[sl][:, None], np.float32(20.0), np.float32(0.0))
        m20 = np.broadcast_to(m20[:, None, :], (IMGS_PER_CORE, 128, OW1)).copy()
        in_maps.append(
            {
                "xp": np.ascontiguousarray(xp),
                "w1l": w1l.astype(NP_STORE),
                "w2l": w2l.astype(NP_STORE),
                "mask20": m20.astype(np.float32),
                "s1": s1t, "t1": t1t, "s2": s2t, "t2": t2t,
            }
        )
    return in_maps


LAST_RESULTS = None


def kernel(**inputs):
    global LAST_RESULTS
    nc = _get_nc()
    in_maps = _prep_core_inputs(**{k: np.asarray(v) for k, v in inputs.items()})
    trace = bool(int(os.environ.get("KTRACE", "0")))
    res = run_bass_kernel_spmd(nc, in_maps, core_ids=list(range(N_CORES)), trace=trace)
    LAST_RESULTS = res
    out = np.concatenate([res.results[i]["out"] for i in range(N_CORES)], axis=0)
    return out.astype(np.float32)
